# revision 1
# baseline (speedup 1.0000x reference)
"""Trainium2 Bass kernel for the CPC/moe_routing problem.

Category-sharded SPMD: 16 categories across 8 cores, 2 per core (paired
big+small by count so the compiled per-slot capacities P0 >= P1 are tight).
Each core, for its rows only:
  f_x = relu(x@W1+b1)@W2+b2 (second layer host-fused with w_s[cat]),
  f_z = Wz^T z'   (z' host-shifted so the bias is exact and pad rows give 0),
  u = f_x @ w_s[cat],  M = u @ f_z^T per category,
  neg_T = row-mean relu(M) (softplus~=relu, |M| large), T = softplus(u.f_z),
  out = log(T+eps) - log(neg_T+eps)  (exact piecewise log-softplus).

Perf structure (vs the 50us baseline):
- 7 big DMA instructions instead of 26 (issue cost ~600ns each on the two
  HWDGE engines), issued at the top of the body on both queues.
- PE warm-up reads a persistent-pool buffer so no SBUF-reuse dependency
  blocks the x/z DMA issue; warm-up length covers the DMA arrival window
  and the 3us HAM clock ramp, then the real matmul stream starts with no
  gap (PE stays at 2.4 GHz).
- pos term = colsum of q = u*fzh (fp16, pool engine) via tiny N=1 matmuls
  interleaved into the neg phase; no f32r prod pass, no second f_z copy.
- relu split ACT/DVE per h-chunk; neg relu-row-sums alternate DVE/ACT with
  accum_out; gpsimd takes the SBUF-side glue (q, pos-chain scalar ops).
- PSUM fits 8 banks exactly: warm(1)->reused, pfz(3), ph(2x2), pu(1), then
  the C phase reuses all: pm(2x2), junk(2), pspos(1).
"""

import math
from contextlib import ExitStack

import numpy as np

import concourse.bass as bass
import concourse.mybir as mybir
import concourse.tile as tile
from concourse import bacc
from concourse import bass_utils
from concourse import hw_specs as _hw_specs

# All activation funcs used here (Relu/Copy/Abs/Exp/Ln) live in the single
# "natural_log_exp_and_others" table set, but the greedy table-load pass
# would pick exp_and_others first and then swap to natural_log mid-kernel
# (1.28us on the ACT engine, on the critical path of the output tail).
# Restrict the pass's choices to the one covering set; ids stay original.
_MONO_TABLE = "natural_log_exp_and_others"


def _mono_tables(arch):
    tabs = _hw_specs.get_activation_tables(arch)
    if _MONO_TABLE not in tabs:
        return tabs
    return {k: (v if k == _MONO_TABLE else set()) for k, v in tabs.items()}


bacc.get_activation_tables = _mono_tables

F32 = mybir.dt.float32
BF16 = mybir.dt.bfloat16
FP16 = mybir.dt.float16
AF = mybir.ActivationFunctionType
ALU = mybir.AluOpType

N, D_IN, HID, Z, C = 8192, 256, 512, 128, 16
N_CORES = 8
EPS32 = float(np.float32(1e-16))
LNEPS = float(np.log(np.float64(np.float32(1e-16))))  # -36.8413614...
POS_THRESH = -9.0
N_WARMUP_MM = 30


def _tiles(start, total, step):
    out = []
    s = 0
    while s < total:
        nt = min(step, total - s)
        out.append((start + s, nt))
        s += nt
    return out


def build_program(P0, P1):
    NCH0, NCH1 = P0 // 128, P1 // 128
    R = P0 + P1
    F = NCH0 + NCH1
    PS = (P0, P1)
    SOFF = (0, P0)
    NCHS = (NCH0, NCH1)
    K = 7 + F  # consts cols: b1[4], b2c[2], eps[1], invd[F]
    ICOL = 7

    nc = bacc.Bacc(
        "TRN2",
        target_bir_lowering=False,
        debug=False,
        enable_asserts=False,
        num_devices=N_CORES,
    )

    xT = nc.dram_tensor("xT", [128, 2, R], FP16, kind="ExternalInput")
    zT = nc.dram_tensor("zT", [128, R], FP16, kind="ExternalInput")
    wzd = nc.dram_tensor("wzd", [128, 128], FP16, kind="ExternalInput")
    w1d = nc.dram_tensor("w1d", [128, 2 * HID], FP16, kind="ExternalInput")
    w2c = nc.dram_tensor("w2c", [128, 2, 4, Z], FP16, kind="ExternalInput")
    cst = nc.dram_tensor("cst", [128, K], F32, kind="ExternalInput")
    outd = nc.dram_tensor("out", [128, F], F32, kind="ExternalOutput")

    with tile.TileContext(nc) as tc, ExitStack() as ctx:
        perm = ctx.enter_context(tc.tile_pool(name="perm", bufs=1))

        # ---- persistent SBUF ----
        sbxt = perm.tile([128, 2, R], FP16)
        sbzt = perm.tile([128, R], FP16)
        sbwz = perm.tile([128, 128], FP16)
        sbw1 = perm.tile([128, 2 * HID], FP16)
        sbw2c = perm.tile([128, 2, 4, Z], FP16)
        sbcst = perm.tile([128, K], F32)
        sbfzh = perm.tile([128, R], FP16)
        sbu = perm.tile([128, R], FP16)
        sbq = perm.tile([128, R], FP16)
        sbht = perm.tile([128, 2, 4, 256], FP16)  # double-buffered relu out
        nacc = perm.tile([128, F], F32)
        junkD = perm.tile([128, P0], FP16)
        junkA = perm.tile([128, P0], FP16)
        sbones = perm.tile([128, 1], FP16)
        wdum = perm.tile([128, 128], BF16)

        # ---- DMAs first, ordered by first use.  sync carries wz/z/x (its
        # DGE starts promptly); scalar carries W1/w2c (its DGE start is
        # delayed ~1.3us by the act-table load); gpsimd SWDGE takes the
        # tiny consts so the HWDGE queues stay short. ----
        nc.sync.dma_start(sbwz[:], wzd[:])
        nc.sync.dma_start(sbzt[:, 0:P0], zT[:, 0:P0])
        nc.sync.dma_start(sbzt[:, P0:R], zT[:, P0:R])
        nc.sync.dma_start(sbxt[:, :, 0:256], xT[:, :, 0:256])
        nc.sync.dma_start(sbxt[:, :, 256:P0], xT[:, :, 256:P0])
        nc.sync.dma_start(sbxt[:, :, P0:R], xT[:, :, P0:R])
        nc.scalar.dma_start(sbw1[:], w1d[:])
        nc.scalar.dma_start(sbw2c[:], w2c[:])

        nc.gpsimd.memset(wdum[:], 0.5)
        nc.gpsimd.memset(sbones[:], 1.0)
        nc.gpsimd.dma_start(sbcst[:], cst[:])

        sbWz = sbwz[:]

        def sbW1(f, h):
            s = f * HID + h * 128
            return sbw1[:, s : s + 128]

        # ---- PE warm-up (HAM clock ramp; covers the DMA arrival window).
        # The pool stays open through stage B so filler matmuls can keep
        # the PE busy-streak alive across data-wait gaps (an idle PE drops
        # back to 1.2 GHz and needs 3us of continuous work to re-ramp). ----
        pswarm = ctx.enter_context(tc.tile_pool(name="pswarm", bufs=1, space="PSUM"))
        pdum = pswarm.tile([16, 128], F32)

        def filler(n, cols=128, after=None):
            for _ in range(n):
                fi = nc.tensor.matmul(
                    pdum[:, 0:cols], wdum[:, 0:16], wdum[:, 0:cols],
                    start=True, stop=True,
                )
                if after is not None:
                    tile.add_dep_helper(fi.ins, after.ins, sync=False,
                                        reason="pin filler")

        filler(N_WARMUP_MM)

        with tc.tile_pool(name="pfzp", bufs=1, space="PSUM") as pfzp:
            # ---- f_z for both slots up front (frees its banks for ph) ----
            pfzA = pfzp.tile([128, P0], F32, tag="pfz", name="pfzA")
            for (ts, nt) in _tiles(0, P0, 512):
                nc.tensor.matmul(
                    pfzA[:, ts : ts + nt], sbWz, sbzt[:, ts : ts + nt],
                    start=True, stop=True,
                )
            nc.scalar.activation(sbfzh[:, 0:P0], pfzA[:], AF.Copy)
            filler(4)
            pfzB = pfzp.tile([128, P1], F32, tag="pfz", name="pfzB")
            for (ts, nt) in _tiles(0, P1, 512):
                nc.tensor.matmul(
                    pfzB[:, ts : ts + nt], sbWz, sbzt[:, P0 + ts : P0 + ts + nt],
                    start=True, stop=True,
                )
            nc.scalar.activation(sbfzh[:, P0:R], pfzB[:], AF.Copy)
            filler(6)

        with (
            tc.tile_pool(name="php", bufs=2, space="PSUM") as php,
            tc.tile_pool(name="pup", bufs=1, space="PSUM") as pup,
        ):

            # ---- MLP over column tiles; one-tile lookahead on PE ----
            tiles = []
            for s in range(2):
                tiles += [(s, ts, nt) for (ts, nt) in _tiles(SOFF[s], PS[s], 256)]
            NT = len(tiles)

            ph_of = {}
            ht_of = {}

            def emit_l1(i):
                s, ts, nt = tiles[i]
                ph = php.tile([128, 4, nt], F32, tag="ph", name=f"ph_{i}")
                ph_of[i] = ph
                for h in range(4):
                    for f in range(2):
                        nc.tensor.matmul(
                            ph[:, h, :],
                            sbW1(f, h),
                            sbxt[:, f, ts : ts + nt],
                            start=(f == 0),
                            stop=(f == 1),
                        )

            def emit_relu(i):
                s, ts, nt = tiles[i]
                ph = ph_of[i]
                ht = sbht[:, i % 2, :, 0:nt]
                ht_of[i] = ht
                for h in range(4):
                    b1h = sbcst[:, h : h + 1]
                    if h < 2:
                        nc.scalar.activation(ht[:, h, :], ph[:, h, :], AF.Relu, bias=b1h)
                    else:
                        nc.vector.tensor_scalar(
                            ht[:, h, :], ph[:, h, :], b1h, 0.0,
                            op0=ALU.add, op1=ALU.max,
                        )

            # group consecutive same-slot tiles into <=512-col psum banks:
            # one u-add per group instead of per tile.
            groups = []
            for i, (s, ts, nt) in enumerate(tiles):
                if groups and groups[-1][0] == s and groups[-1][2] + nt <= 512:
                    groups[-1][1].append(i)
                    groups[-1][2] += nt
                else:
                    groups.append([s, [i], nt])
            grp_of = {i: g for g in range(len(groups)) for i in groups[g][1]}
            pu_of = {}

            def emit_l2(i):
                s, ts, nt = tiles[i]
                ht = ht_of[i]
                g = grp_of[i]
                _, members, gw = groups[g]
                if i == members[0]:
                    pu_of[g] = pup.tile([128, gw], F32, tag="pu", name=f"pu_{g}")
                off = tiles[members[0]][1]
                pu = pu_of[g][:, ts - off : ts - off + nt]
                for q in range(4):
                    nc.tensor.matmul(
                        pu,
                        sbw2c[:, s, q, :],
                        ht[:, q, :],
                        start=(q == 0),
                        stop=(q == 3),
                    )
                if i == members[-1]:
                    nc.vector.tensor_scalar_add(
                        sbu[:, off : off + gw], pu_of[g][:],
                        sbcst[:, 4 + s : 5 + s],
                    )

            emit_l1(0)
            emit_relu(0)
            for i in range(1, NT):
                emit_l1(i)
                emit_relu(i)
                emit_l2(i - 1)
            emit_l2(NT - 1)

            # q = u * fzh per slot on the pool engine (SBUF-only)
            nc.gpsimd.tensor_tensor(
                sbq[:, 0:P0], sbu[:, 0:P0], sbfzh[:, 0:P0], op=ALU.mult
            )
            nc.gpsimd.tensor_tensor(
                sbq[:, P0:R], sbu[:, P0:R], sbfzh[:, P0:R], op=ALU.mult
            )

        # ======== Stage C: neg sums + pos columns ========
        with (
            tc.tile_pool(name="psm", bufs=3, space="PSUM") as psm,
            tc.tile_pool(name="pspp", bufs=1, space="PSUM") as pspp,
        ):
            pspos = pspp.tile([128, 16], F32)

            blocks = [(0, ic) for ic in range(NCH0)] + [(1, ic) for ic in range(NCH1)]

            def emit_pos(col):
                c0 = col * 128
                nc.tensor.matmul(
                    pspos[:, col : col + 1],
                    sbq[:, c0 : c0 + 128],
                    sbones[:],
                    start=True, stop=True,
                )

            for b, (s, ic) in enumerate(blocks):
                ucol = SOFF[s] + ic * 128
                pmt = psm.tile([128, P0], F32, tag="pm", name=f"pm_{b}")
                pm = pmt[:, 0 : PS[s]]
                last_mm = None
                for (ts, nt) in _tiles(SOFF[s], PS[s], 512):
                    last_mm = nc.tensor.matmul(
                        pm[:, ts - SOFF[s] : ts - SOFF[s] + nt],
                        sbu[:, ucol : ucol + 128],
                        sbfzh[:, ts : ts + nt],
                        start=True, stop=True,
                    )
                # slot-0 pos columns ride along during slot-1 neg blocks;
                # slot-1 pos columns at the end.  Fillers (pinned after this
                # block's matmul) keep the PE busy-streak -- and so the
                # 2.4 GHz clock -- alive while pm buffers recycle.
                if s == 1:
                    emit_pos(ic)
                filler(3, cols=64, after=last_mm)
                col = NCH0 + ic if s == 1 else ic
                if b % 2 == 0:
                    nc.vector.tensor_scalar(
                        junkD[:, 0 : PS[s]], pm[:], 0.0, 0.0,
                        op0=ALU.max, op1=ALU.add,
                        accum_out=nacc[:, col : col + 1],
                    )
                else:
                    nc.scalar.activation(
                        junkA[:, 0 : PS[s]], pm[:], AF.Relu,
                        accum_out=nacc[:, col : col + 1],
                    )
            for ic in range(NCH1, NCH0):
                emit_pos(ic)
            for ic in range(NCH1):
                emit_pos(NCH0 + ic)

            # ======== tail: neg log + exact piecewise log-softplus(pos) ======
            vec = ctx.enter_context(tc.tile_pool(name="vec", bufs=1))
            sbeps = sbcst[:, 6:7]

            t_neg = vec.tile([128, F], F32)
            nc.vector.tensor_mul(t_neg[:], nacc[:], sbcst[:, ICOL : ICOL + F])
            t_lnneg = vec.tile([128, F], F32)
            i_lnneg = nc.scalar.activation(t_lnneg[:], t_neg[:], AF.Ln, bias=sbeps)

            tpos = vec.tile([128, F], F32)
            nc.vector.tensor_copy(tpos[:], pspos[:, 0:F])

            t_ax = vec.tile([128, F], F32)
            i_ax = nc.scalar.activation(t_ax[:], tpos[:], AF.Abs)
            t_y = vec.tile([128, F], F32)
            nc.vector.tensor_scalar_add(t_y[:], tpos[:], -LNEPS)
            t_ay = vec.tile([128, F], F32)
            i_ay = nc.scalar.activation(t_ay[:], t_y[:], AF.Abs)
            t_e2 = vec.tile([128, F], F32)
            i_e2 = nc.scalar.activation(t_e2[:], t_ax[:], AF.Exp, scale=-1.0)
            t_e1 = vec.tile([128, F], F32)
            i_e1 = nc.scalar.activation(t_e1[:], t_ay[:], AF.Exp, scale=-1.0)
            t_r2 = vec.tile([128, F], F32)
            nc.vector.tensor_scalar_max(t_r2[:], tpos[:], 0.0)
            t_r1 = vec.tile([128, F], F32)
            nc.vector.tensor_scalar_max(t_r1[:], t_y[:], 0.0)
            t_l2 = vec.tile([128, F], F32)
            i_l2 = nc.scalar.activation(t_l2[:], t_e2[:], AF.Ln, bias=1.0)
            t_l1 = vec.tile([128, F], F32)
            i_l1 = nc.scalar.activation(t_l1[:], t_e1[:], AF.Ln, bias=1.0)
            t_sp = vec.tile([128, F], F32)
            nc.vector.tensor_add(t_sp[:], t_r2[:], t_l2[:])
            t_p2 = vec.tile([128, F], F32)
            i_p2 = nc.scalar.activation(t_p2[:], t_sp[:], AF.Ln, bias=sbeps)
            t_p1 = vec.tile([128, F], F32)
            nc.vector.scalar_tensor_tensor(
                t_p1[:], t_r1[:], LNEPS, t_l1[:], op0=ALU.add, op1=ALU.add
            )
            t_m = vec.tile([128, F], mybir.dt.int32)
            nc.vector.tensor_scalar(t_m[:], tpos[:], POS_THRESH, None, op0=ALU.is_lt)
            t_posln = vec.tile([128, F], F32)
            nc.vector.select(t_posln[:], t_m[:], t_p1[:], t_p2[:])

            t_out = vec.tile([128, F], F32)
            nc.vector.tensor_sub(t_out[:], t_posln[:], t_lnneg[:])
            nc.sync.dma_start(outd[:], t_out[:])

    nc.compile()
    return nc


def prepare(x, c, z, W1, b1, W2, b2, Wz, bz, w_s):
    x = np.ascontiguousarray(np.asarray(x, dtype=np.float32))
    z = np.ascontiguousarray(np.asarray(z, dtype=np.float32))
    W1 = np.asarray(W1, dtype=np.float32)
    b1 = np.asarray(b1, dtype=np.float32)
    W2 = np.asarray(W2, dtype=np.float32)
    b2 = np.asarray(b2, dtype=np.float32)
    Wz = np.asarray(Wz, dtype=np.float32)
    bz = np.asarray(bz, dtype=np.float32)
    w_s = np.asarray(w_s, dtype=np.float32)
    ci = np.asarray(c).astype(np.int64)

    idx = [np.nonzero(ci == g)[0] for g in range(C)]
    cnt = np.array([len(i) for i in idx])
    order = np.argsort(-cnt, kind="stable")
    # core k gets (order[k], order[15-k]); slot capacities from the global
    # extremes so the same NEFF fits every core tightly.
    P0 = 128 * max(1, math.ceil(cnt[order[0]] / 128))
    P1 = 128 * max(1, math.ceil(cnt[order[N_CORES]] / 128))
    NCH0, NCH1 = P0 // 128, P1 // 128
    R = P0 + P1
    F = NCH0 + NCH1
    K = 7 + F

    # z' = z - z0 so that Wz16^T z' = Wz^T z + bz exactly on device; pad
    # rows use z'=0 giving f_z = 0 exactly.
    Wz16 = Wz.astype(np.float16).astype(np.float64)
    z0 = np.linalg.solve(Wz16.T, -bz.astype(np.float64)).astype(np.float32)

    W1h = np.ascontiguousarray(
        W1.reshape(2, 128, HID).transpose(1, 0, 2).reshape(128, 2 * HID)
    ).astype(np.float16)
    Wzh = np.ascontiguousarray(Wz.astype(np.float16))

    W2c_all = np.einsum(
        "hd,cde->che", W2.astype(np.float64), w_s.astype(np.float64)
    )  # [C, HID, Z]
    b2c_all = np.einsum("d,cde->ce", b2.astype(np.float64), w_s.astype(np.float64))

    in_maps = []
    slots = []
    for k in range(N_CORES):
        cats = (int(order[k]), int(order[2 * N_CORES - 1 - k]))
        caps = (P0, P1)
        rows = []
        padf = []
        inv = np.zeros((128, F), dtype=np.float32)
        colbase = 0
        for s, g in enumerate(cats):
            n = cnt[g]
            fill = idx[g][0] if n > 0 else 0
            rows.append(
                np.concatenate([idx[g], np.full(caps[s] - n, fill, dtype=np.int64)])
            )
            pf = np.zeros(caps[s], dtype=bool)
            pf[n:] = True
            padf.append(pf)
            nch = caps[s] // 128
            inv[:, colbase : colbase + nch] = 1.0 / max(n, 1)
            colbase += nch
        rows = np.concatenate(rows)
        padf = np.concatenate(padf)

        xk = x[rows]  # [R, 256]
        xTk = np.ascontiguousarray(
            xk.T.reshape(2, 128, R).transpose(1, 0, 2)
        ).astype(np.float16)  # [128, 2, R]
        zk = z[rows] - z0[None, :]
        zk[padf] = 0.0
        zTk = np.ascontiguousarray(zk.T).astype(np.float16)  # [128, R]

        w2ck = np.zeros((128, 2, 4, Z), dtype=np.float16)
        for s, g in enumerate(cats):
            w2ck[:, s] = (
                W2c_all[g].reshape(4, 128, Z).transpose(1, 0, 2).astype(np.float16)
            )

        cstk = np.zeros((128, K), dtype=np.float32)
        cstk[:, 0:4] = b1.reshape(4, 128).T
        for s, g in enumerate(cats):
            cstk[:, 4 + s] = b2c_all[g].astype(np.float32)
        cstk[:, 6] = EPS32
        cstk[:, 7 : 7 + F] = inv

        in_maps.append(
            {"xT": xTk, "zT": zTk, "wzd": Wzh, "w1d": W1h, "w2c": w2ck, "cst": cstk}
        )
        slots.append((cats, [int(cnt[g]) for g in cats]))
    return P0, P1, in_maps, slots, idx


def gather_output(P0, P1, slots, idx, core_outs):
    NCH0, NCH1 = P0 // 128, P1 // 128
    out_full = np.zeros(N, dtype=np.float32)
    for k in range(N_CORES):
        om = core_outs[k]  # [128, F]; out[p, colbase+ic] = row soff + ic*128 + p
        cats, counts = slots[k]
        colbase = 0
        for s, g in enumerate(cats):
            nch = (NCH0, NCH1)[s]
            rows_cat = om[:, colbase : colbase + nch].T.reshape(nch * 128)
            n = counts[s]
            if n:
                out_full[idx[g]] = rows_cat[:n]
            colbase += nch
    return out_full


def kernel(x, c, z, W1, b1, W2, b2, Wz, bz, w_s):
    P0, P1, in_maps, slots, idx = prepare(x, c, z, W1, b1, W2, b2, Wz, bz, w_s)
    nc = build_program(P0, P1)
    res = bass_utils.run_bass_kernel_spmd(nc, in_maps, core_ids=list(range(N_CORES)))
    return gather_output(P0, P1, slots, idx, [r["out"] for r in res.results])

